# revision 6
# baseline (speedup 1.0000x reference)
"""Trainium2 kernel for ChannelQuadLayer.

Per-pixel quadratic channel expansion + 1x1 conv:
    quad = x[:, ii] * x[:, jj]  (all 2080 upper-tri channel pairs)
    y    = concat([x, quad])    -> [B, 2144, H, W]
    out  = einsum('bchw,oc->bohw', y, fc_w)

Strategy (8 NeuronCores, batch-parallel, one sample per core):
  * The 2080 unordered channel pairs are exactly the cyclic diagonals
    d=0..32 of the 64-channel index ring: pairs {i, (i+d)%64}.
  * Host prepares 9 "rotation buffers" B_k = [roll(x,-t_k); roll(x,-u_k)]
    (128 partitions x 4096 pixels). A single elementwise multiply of two
    such buffers yields TWO complete cyclic diagonals (top half: diagonal
    t_j - t_i, bottom half: u_j - u_i). A difference cover (found by
    search) produces all diagonals 1..32 in 16 multiplies; diagonal 0
    (squares) comes from one ScalarE Square op. All vector ops use the
    full 128 partitions with 0-based alignment.
  * y-rows: 64 linear + 64 squares + 16*128 pair rows = 2176 = 17*128,
    an exact 17-chunk contraction. fc_w is permuted/padded to this row
    order on the host (duplicate pair rows get zero weight).
  * GEMM: out[256, 4096] = Wt[2176, 256]^T @ y[2176, 4096] on TensorE
    in float32r (full-rate fp32, ~1.5e-4 rel err), accumulating 17
    chunks into PSUM, k-outer so each y chunk is consumed right after
    its producer; PSUM tiles DMA straight to HBM.
"""

import sys

sys.path.insert(0, "/opt/trn_rl_repo")

import numpy as np

import concourse.bass as bass
import concourse.tile as tile
from concourse import bacc, mybir
from concourse.bass_utils import run_bass_kernel_spmd

B, C, H, W = 8, 64, 64, 64
PIX = H * W  # 4096
OUT = 256
NCORES = 8

# rotation difference cover: ops (i,j) give diagonals D(t_j-t_i) (top half)
# and D(u_j-u_i) (bottom half); together exactly {1..32}.
T_ROT = [0, 8, 22, 24, 42, 48, 49, 57, 60]
U_ROT = [0, 59, 16, 38, 55, 22, 30, 54, 35]
OPS = [(0, 7), (1, 3), (1, 4), (1, 6), (2, 3), (2, 4), (2, 6), (2, 8),
       (3, 4), (3, 8), (4, 5), (4, 7), (5, 7), (5, 8), (6, 7), (6, 8)]
NB = len(T_ROT)        # 9 rotation buffers
KCH = 1 + len(OPS)     # 17 contraction chunks of 128 rows
FD = 1024              # pixel tile width
NPASS = PIX // FD      # 4
NT = FD // 512         # 512-wide matmul slices per pass

F32 = mybir.dt.float32
F32R = mybir.dt.float32r


def row_pairs():
    """Channel pair (c1, c2) for every global y row, or ('lin', c)."""
    rows = []
    for p in range(128):  # chunk 0
        rows.append(("lin", p) if p < 64 else (p - 64, p - 64))
    for (i, j) in OPS:
        for p in range(128):
            if p < 64:
                c1, c2 = (p + T_ROT[i]) % 64, (p + T_ROT[j]) % 64
            else:
                c1, c2 = (p - 64 + U_ROT[i]) % 64, (p - 64 + U_ROT[j]) % 64
            rows.append((min(c1, c2), max(c1, c2)))
    return rows


def build_wt(fc_w):
    """Permute fc_w [OUT, 2144] into Wt [KCH, 128, OUT] matching y rows."""
    ii, jj = np.triu_indices(C)
    pair2col = {(a, b): C + k for k, (a, b) in enumerate(zip(ii, jj))}
    wt = np.zeros((KCH * 128, OUT), np.float32)
    seen = set()
    for g, r in enumerate(row_pairs()):
        if r[0] == "lin":
            wt[g] = fc_w[:, r[1]]
        elif r not in seen:
            seen.add(r)
            wt[g] = fc_w[:, pair2col[r]]
    assert len(seen) == C * (C + 1) // 2
    return np.ascontiguousarray(wt.reshape(KCH, 128, OUT))


_cached = None


def _build_module():
    global _cached
    if _cached is not None:
        return _cached
    nc = bacc.Bacc("TRN2", target_bir_lowering=False, debug=False,
                   num_devices=NCORES)
    b_d = [nc.dram_tensor(f"b{i}", [128, PIX], F32R, kind="ExternalInput")
           for i in range(NB)]
    wt_d = nc.dram_tensor("wt", [KCH, 128, OUT], F32R, kind="ExternalInput")
    out_d = nc.dram_tensor("out", [OUT, PIX], F32, kind="ExternalOutput")

    with tile.TileContext(nc) as tc:
        with tc.tile_pool(name="wt", bufs=1) as wt_pool, \
             tc.tile_pool(name="bsrc", bufs=2) as b_pool, \
             tc.tile_pool(name="y", bufs=4) as y_pool, \
             tc.tile_pool(name="ostage", bufs=4) as o_pool, \
             tc.tile_pool(name="psum", bufs=8, space="PSUM") as ps_pool:

            wt_t = []
            for k in range(KCH):
                t = wt_pool.tile([128, OUT], F32R, tag=f"wt{k}")
                nc.sync.dma_start(t[:], wt_d.ap()[k])
                wt_t.append(t)

            for ps in range(NPASS):
                fs = slice(ps * FD, (ps + 1) * FD)
                bt = []
                for i in range(NB):
                    t = b_pool.tile([128, FD], F32R, tag=f"b{i}")
                    nc.sync.dma_start(t[:], b_d[i].ap()[:, fs])
                    bt.append(t)

                psum = [ps_pool.tile([128, 512], F32, tag="ps", name=f"ps{ps}_{g}")
                        for g in range(2 * NT)]

                for k in range(KCH):
                    yk = y_pool.tile([128, FD], F32R, tag="y")
                    if k == 0:
                        # linear rows + squares
                        nc.sync.dma_start(yk[0:64, :], b_d[0].ap()[0:64, fs])
                        nc.scalar.activation(
                            yk[64:128, :], bt[0][64:128, :],
                            mybir.ActivationFunctionType.Square)
                    else:
                        i, j = OPS[k - 1]
                        nc.vector.tensor_mul(yk[:], bt[i][:], bt[j][:])
                    for m in range(2):
                        lhsT = wt_t[k][:, m * 128:(m + 1) * 128]
                        for n in range(NT):
                            nc.tensor.matmul(
                                psum[m * NT + n][:],
                                lhsT,
                                yk[:, n * 512:(n + 1) * 512],
                                start=(k == 0), stop=(k == KCH - 1))

                for m in range(2):
                    for n in range(NT):
                        ot = o_pool.tile([128, 512], F32, tag="ostage",
                                         name=f"o{ps}_{m}_{n}")
                        nc.scalar.activation(
                            ot[:], psum[m * NT + n][:],
                            mybir.ActivationFunctionType.Identity)
                        nc.sync.dma_start(
                            out_d.ap()[m * 128:(m + 1) * 128,
                                       ps * FD + n * 512:ps * FD + (n + 1) * 512],
                            ot[:])
    nc.compile()
    _cached = nc
    return nc


def kernel(x, fc_w):
    x = np.asarray(x, dtype=np.float32)
    fc_w = np.asarray(fc_w, dtype=np.float32)
    nc = _build_module()
    wt = build_wt(fc_w)

    in_maps = []
    for b in range(B):
        xc = np.ascontiguousarray(x[b].reshape(C, PIX))
        m = {"wt": wt}
        for i in range(NB):
            m[f"b{i}"] = np.ascontiguousarray(np.concatenate(
                [np.roll(xc, -T_ROT[i], axis=0), np.roll(xc, -U_ROT[i], axis=0)]))
        in_maps.append(m)

    res = run_bass_kernel_spmd(nc, in_maps, list(range(NCORES)))
    out = np.stack([res.results[b]["out"].reshape(OUT, H, W) for b in range(B)])
    return out
